# revision 58
# baseline (speedup 1.0000x reference)
"""AdapLSNet MLP kernel for 8 TRN2 NeuronCores (data-parallel).

reference:
    h  = elu(x @ W0 + b0)
    h  = elu(h @ W1 + b1)
    out = sigmoid(h @ W2 + b2)          # [B, 1]
    alpha = piecewise(out)               # a=0.1, b=0.2, c=0.8
    returns (out, alpha)

Strategy
- Shard batch (32768) across 8 cores (4096 rows each); replicate weights.
- Host pre-transposes each x shard to x^T so every layer's activations
  live in [feature(partitions), batch(free)] layout; no on-device
  transposes.  Per-chunk xt is stored as one [128, 4096] block (8KB
  per-partition contiguous runs -> large DMA packets).
- L1/L2 in fp16 (full PE rate, FWL weight loads, half the DMA/SBUF
  bytes; measured end-to-end rel err 1.0e-3 vs the 2e-2 gate).  fp8
  DoubleRow was evaluated and rejected: alpha has only ~8 nonzero tail
  entries, and fp8 noise on `out` gives alpha rel err 0.12-0.24.
- Single fused pass: W0 (fp16) and W1 (fp16) are SBUF-resident, so h1
  never leaves the chip.
- Startup is chip-HBM-contention-bound (~150-250 GB/s per core while
  all 8 cores pull their weights), so the DMA stream is ordered by
  first-use with minimal first-need bytes: b0/b1/b2/w2 first (b0 gates
  every L1 ScalarE activation - emitting it last deadlocked the psum
  ring for 23us and triggered a 48us HAM half-clock window); then the
  startup chunks xt0-2 in a k-block layout (eight [128,512] blocks per
  chunk, k order) interleaved pairwise with W0's first 128-col
  half-strip so L1 starts ~12us in and its k-loop PACES with block
  arrivals - any DMA wait appears as sub-3.4us micro-gaps that never
  re-throttle the HAM clock gate, instead of one big stall; then W0
  strip-major in 256-col strips (strip s of every slab before strip
  s+1, matching L1 m-tile consumption), then W1 as full slabs split
  into partition quarters (4KB runs).  Steady-state chunks use a wide
  [128, 4096] per-chunk block (8KB per-partition runs).  Transfers
  alternate the sync (HWDGE) and gpsimd (SWDGE) queue families.
- Software pipeline: L1 runs three batch-chunks ahead of L2 so the PE
  has L1 work while W1 streams in at startup.
- L3 (h2 @ W2, M=1) runs OFF the PE: per m-tile a single DVE
  scalar_tensor_tensor accumulates acc += w2[:,m] (x) h2 (per-partition
  scalar multiply) in two parity chains, and two ones-vector matmuls
  per chunk reduce the 128 partials -> z3 [1,512].  This frees 16 full
  512-col MM slots per chunk (~26us of PE time total) vs packed
  matmuls.
- elu(z) = min(exp(z) - 1, relu(z)): 2 ScalarE LUT ops reading PSUM with
  the bias fused + 1 fused VectorE (e-1) min r op; the last m-tile's
  relu runs on DVE in parallel with the ScalarE exp to shorten the
  end-of-chunk serial chain.
- alpha = relu(-0.5*out + 0.1) + relu(0.5*out - 0.4); the branches are
  mutually exclusive for out in [0,1], so it is computed as
  relu(|0.5*out - 0.25| - 0.15): 2 ScalarE ops, no DVE combine.
- The final chunk runs as two 256-col halves so most of its epilogue
  hides under the second half's matmuls.
- PE warmup matmuls keep the HAM clock gate released (2.4 GHz) across
  the initial DMA wait; NWARM is sized so warmup ends right as the
  first L1 inputs land (a >3.4us PE gap would re-throttle to 1.2 GHz).

Measured: 780.6us (baseline) -> ~696us on 8 axon trn2 cores,
rel err 1.1e-3 (gate 2e-2), ~94% of the fp16 PE roofline (656us).
"""

import numpy as np

BATCH = 32768
DIN = 1024
DH = 2048
NCORES = 8
SHARD = BATCH // NCORES          # 4096
CHUNK = 512
NCH = SHARD // CHUNK             # 8
KI = DIN // 128                  # 8
KH = DH // 128                   # 16
MH = DH // 128                   # 16
NH1S = 48                        # h1 slots (fp16 [128,512], 1KB each; 3 chunks)
NXTC = 4                         # xt chunk-tile ring ([128,4096] fp16, 8KB/part)
NWARM = 26                       # PE warmup matmuls (HAM un-throttle; sized
                                 # to end ~14us, inside the DMA-paced L1(0)
                                 # phase, so slow-DMA runs keep enough PE
                                 # activity in the HAM window to avoid a
                                 # half-clock blip; L1's first m-tile is
                                 # block-arrival-bound either way)
W0NS = 8                         # W0 strips per slab (256 cols)


def _install_profile_shim():
    """Allow trace=True under axon (exec_time_ns capture) if possible."""
    import sys
    import types

    try:
        import antenv

        if "antenv.axon_hooks" in sys.modules:
            return
        mod = types.ModuleType("antenv.axon_hooks")
        _hook = [None]
        mod.set_axon_ntff_profile_hook = lambda h: _hook.__setitem__(0, h)
        mod.get_axon_ntff_profile_hook = lambda: _hook[0]
        sys.modules["antenv.axon_hooks"] = mod
        antenv.axon_hooks = mod
        try:
            from trn_agent_boot.trn_boot import _ntff_profile_via_ctypes

            mod.set_axon_ntff_profile_hook(
                _ntff_profile_via_ctypes("/opt/axon/libaxon_pjrt.so")
            )
        except Exception:
            pass
    except Exception:
        pass


_NC_CACHE = None


def _build():
    global _NC_CACHE
    if _NC_CACHE is not None:
        return _NC_CACHE

    import concourse.mybir as mybir
    import concourse.tile as tile
    from concourse import bacc

    F32R = mybir.dt.float32r
    F32 = mybir.dt.float32
    F16 = mybir.dt.float16
    AF = mybir.ActivationFunctionType
    ALU = mybir.AluOpType

    nc = bacc.Bacc("TRN2", target_bir_lowering=False)

    # DMA packet size == per-PARTITION contiguous run length of the SBUF
    # destination; 1KB-run tiles capped the DMA engines at ~90-160 GB/s
    # and starved the startup pipeline.  So transfers below are
    # full-width row slices into wide tiles:
    # xt: chunk-tile blocks [128, KI*CHUNK] (partition p col k*512+c =
    #     xT[k*128+p, n*512+c]) -> 8KB/partition runs
    xt_ext = nc.declare_dram_parameter(
        "xt", [NCH * 128, KI * CHUNK], F16, isOutput=False)
    # first 3 chunks duplicated in k-block layout (row (n*KI+k)*128+p,
    # col c = xT[k*128+p, n*512+c]): the startup chunks stream as eight
    # [128,512] k-blocks each, so L1's k-loop paces with block arrivals
    # (sub-3.4us micro-gaps instead of one big HAM-re-throttling stall)
    xtkb_ext = nc.declare_dram_parameter(
        "xtkb", [3 * KI * 128, CHUNK], F16, isOutput=False)
    # w0: 2 half-width strips per slab: row (s*KI+k)*128+p, col c =
    #     W0[k*128+p, s*1024+c] -> 2KB/partition runs
    w0_ext = nc.declare_dram_parameter(
        "w0", [W0NS * KI * 128, DH // W0NS], F16, isOutput=False)
    # w1: original [DH, DH] layout; full slabs split by partition
    #     quarters -> 4KB/partition runs
    w1_ext = nc.declare_dram_parameter("w1", [DH, DH], F16, isOutput=False)
    w2_ext = nc.declare_dram_parameter("w2", [128, KH], F32, isOutput=False)
    b0_ext = nc.declare_dram_parameter("b0", [128, MH], F32, isOutput=False)
    b1_ext = nc.declare_dram_parameter("b1", [128, MH], F32, isOutput=False)
    b2_ext = nc.declare_dram_parameter("b2", [1, 1], F32, isOutput=False)
    out_ext = nc.declare_dram_parameter("out", [1, SHARD], F32, isOutput=True)
    alpha_ext = nc.declare_dram_parameter("alpha", [1, SHARD], F32, isOutput=True)

    with tile.TileContext(nc) as tc:
        with (
            tc.tile_pool(name="w0p", bufs=1) as w0p,
            tc.tile_pool(name="w1p", bufs=1) as w1p,
            tc.tile_pool(name="xtp", bufs=1) as xtp,
            tc.tile_pool(name="h1p", bufs=1) as h1p,
            tc.tile_pool(name="hpool", bufs=2) as hpool,
            tc.tile_pool(name="h2p", bufs=4) as h2p,
            tc.tile_pool(name="accp", bufs=1) as accp,
            tc.tile_pool(name="redp", bufs=2) as redp,
            tc.tile_pool(name="cst", bufs=1) as cst,
            tc.tile_pool(name="ps", bufs=6, space="PSUM") as ps,
            tc.tile_pool(name="ops", bufs=2, space="PSUM") as ops,
        ):
            w0_sb = [
                w0p.tile([128, DH], F16, tag=f"w0_{k}", name=f"w0_{k}")
                for k in range(KI)
            ]
            w1_sb = [
                w1p.tile([128, DH], F16, tag=f"w1_{k}", name=f"w1_{k}")
                for k in range(KH)
            ]

            def w0_lhsT(k, m):
                return w0_sb[k][:, m * 128:(m + 1) * 128]

            def w1_lhsT(k, m):
                return w1_sb[k][:, m * 128:(m + 1) * 128]

            def emit_xt(n, nsplit=8):
                """One [128, 4096] chunk tile, DMA'd as `nsplit`
                partition-range slices (keeps 8KB/partition packets,
                spreads across queues)."""
                t = xtp.tile([128, KI * CHUNK], F16, tag=f"xtc{n % NXTC}",
                             name=f"xt_{n}")
                rows = 128 // nsplit
                for j in range(nsplit):
                    eng = nc.sync if (j % 2 == 0) else nc.gpsimd
                    eng.dma_start(
                        t[j * rows:(j + 1) * rows, :],
                        xt_ext[n * 128 + j * rows:n * 128 + (j + 1) * rows, :],
                    )
                return t

            # --- small, first-use-critical tensors FIRST: b0 gates every
            # L1 ScalarE activation (and thence psum recycling) ---
            b0_sb = cst.tile([128, MH], F32, tag="b0", name="b0")
            nc.sync.dma_start(b0_sb[:], b0_ext[:])
            b1_sb = cst.tile([128, MH], F32, tag="b1", name="b1")
            nc.sync.dma_start(b1_sb[:], b1_ext[:])
            b2_sb = cst.tile([1, 1], F32, tag="b2", name="b2")
            nc.sync.dma_start(b2_sb[:], b2_ext[:])
            w2_sb = cst.tile([128, KH], F32, tag="w2", name="w2")
            nc.sync.dma_start(w2_sb[:], w2_ext[:])
            # alpha = relu(-0.5*o + 0.1) + relu(0.5*o - 0.4); the two
            # branches are mutually exclusive on o in [0,1], so
            # alpha = relu(|0.5*o - 0.25| - 0.15)  (2 ScalarE ops)
            c_ab = cst.tile([1, 1], F32, tag="c_ab", name="c_ab")
            c_rb = cst.tile([1, 1], F32, tag="c_rb", name="c_rb")
            c_sp = cst.tile([1, 1], F32, tag="c_sp", name="c_sp")
            nc.vector.memset(c_ab[:], -0.25)
            nc.vector.memset(c_rb[:], -0.15)
            nc.vector.memset(c_sp[:], 0.5)
            ones_sb = cst.tile([128, 1], F16, tag="ones", name="ones")
            nc.vector.memset(ones_sb[:], 1.0)

            def emit_xt_kb(n, fam=0):
                """Startup chunks: one [128, 4096] tile filled by eight
                [128, 512] k-block DMAs in k order (matches the L1
                k-loop's consumption order)."""
                t = xtp.tile([128, KI * CHUNK], F16, tag=f"xtc{n % NXTC}",
                             name=f"xt_{n}")
                for k in range(KI):
                    eng = nc.sync if ((k + fam) % 2 == 0) else nc.gpsimd
                    row = (n * KI + k) * 128
                    eng.dma_start(
                        t[:, k * CHUNK:(k + 1) * CHUNK],
                        xtkb_ext[row:row + 128, :],
                    )
                return t

            # --- startup stream, first-use-ordered and k-interleaved:
            # m0's k-step needs (xt0 block k, W0 slab k cols 0-127), so
            # emit those pairwise across the two queue families; L1 can
            # then start ~10us in and pace with arrivals. ---
            xt_tiles = {}
            t0 = xtp.tile([128, KI * CHUNK], F16, tag="xtc0", name="xt_0")
            xt_tiles[0] = t0
            for k in range(KI):
                nc.sync.dma_start(
                    t0[:, k * CHUNK:(k + 1) * CHUNK],
                    xtkb_ext[k * 128:(k + 1) * 128, :],
                )
                nc.gpsimd.dma_start(
                    w0_sb[k][:, 0:128],
                    w0_ext[k * 128:(k + 1) * 128, 0:128],
                )
            for k in range(KI):
                eng = nc.sync if (k % 2 == 1) else nc.gpsimd
                eng.dma_start(
                    w0_sb[k][:, 128:256],
                    w0_ext[k * 128:(k + 1) * 128, 128:256],
                )
            W0S = DH // W0NS         # 256-col strips, strip-major
            for s in range(1, W0NS):
                for k in range(KI):
                    eng = nc.sync if ((s + k) % 2 == 0) else nc.gpsimd
                    row = (s * KI + k) * 128
                    eng.dma_start(
                        w0_sb[k][:, s * W0S:(s + 1) * W0S],
                        w0_ext[row:row + 128, :],
                    )

            xt_tiles[1] = emit_xt_kb(1, fam=0)
            xt_tiles[2] = emit_xt_kb(2, fam=1)

            # --- W1: full slabs as 4 partition quarters [32, 2048]
            # (source rows 128k+32q..+32 are contiguous 128KB) ---
            for k in range(KH):
                for q in range(4):
                    eng = nc.sync if ((k + q) % 2 == 0) else nc.gpsimd
                    r0 = 128 * k + 32 * q
                    eng.dma_start(w1_sb[k][32 * q:32 * q + 32, :],
                                  w1_ext[r0:r0 + 32, :])

            # PE warmup: dependency-free matmuls on a memset tile keep the
            # PE busy during the initial DMA wait so the HAM clock gate is
            # already released (2.4 GHz) when real matmuls start.
            wu = hpool.tile([128, CHUNK], F16, tag="e", name="wu")
            nc.vector.memset(wu[:], 0.0)
            for i in range(NWARM):
                wps = ps.tile([128, CHUNK], F32, tag="ps", name=f"wups_{i}")
                nc.tensor.matmul(
                    wps[:], wu[:, 0:128], wu[:], start=True, stop=True,
                )

            h1_tiles = {}

            def l1_chunk(n):
                """L1: h1(n) = elu(W0.T @ xT(n) + b0), kept in SBUF."""
                xt_sb = xt_tiles.pop(n)
                h1base = (MH * n) % NH1S
                tiles = []
                for m in range(MH):
                    psum = ps.tile([128, CHUNK], F32, tag="ps",
                                   name=f"psA_{n}_{m}")
                    for k in range(KI):
                        nc.tensor.matmul(
                            psum[:], w0_lhsT(k, m),
                            xt_sb[:, k * CHUNK:(k + 1) * CHUNK],
                            start=(k == 0), stop=(k == KI - 1),
                        )
                    e = hpool.tile([128, CHUNK], F32, tag="e", name="e")
                    r = hpool.tile([128, CHUNK], F32, tag="r", name="r")
                    nc.scalar.activation(e[:], psum[:], AF.Exp,
                                         bias=b0_sb[:, m:m + 1])
                    nc.scalar.activation(r[:], psum[:], AF.Relu,
                                         bias=b0_sb[:, m:m + 1])
                    h1 = h1p.tile(
                        [128, CHUNK], F16, tag=f"h{(h1base + m) % NH1S}",
                        name=f"h1_{n}_{m}",
                    )
                    nc.vector.scalar_tensor_tensor(
                        h1[:], e[:], 1.0, r[:], ALU.subtract, ALU.min
                    )
                    tiles.append(h1)
                h1_tiles[n] = tiles

            def l2_chunk(n, c0=0, cw=CHUNK, pop=True, merge=True):
                """L2 + L3 + sigmoid + alpha for cols [c0, c0+cw) of
                chunk n.

                L3 runs off the PE: a DVE scalar_tensor_tensor chain
                accumulates acc += w2[:,m] (x) h2 per m-tile, then one
                ones-vector matmul reduces partitions -> z3 [1,cw].
                The final chunk runs as two halves so most of its
                epilogue hides under the second half's matmuls.
                """
                h1_sb = h1_tiles[n]
                if pop:
                    del h1_tiles[n]
                prev = [None, None]       # even / odd m accumulation chains
                for m in range(MH):
                    psum = ps.tile([128, cw], F32, tag="ps",
                                   name=f"psB_{n}_{m}_{c0}")
                    for k in range(KH):
                        nc.tensor.matmul(
                            psum[:], w1_lhsT(k, m),
                            h1_sb[k][:, c0:c0 + cw],
                            start=(k == 0), stop=(k == KH - 1),
                        )
                    e = hpool.tile([128, cw], F32, tag="e", name="e")
                    r = hpool.tile([128, cw], F32, tag="r", name="r")
                    nc.scalar.activation(e[:], psum[:], AF.Exp,
                                         bias=b1_sb[:, m:m + 1])
                    if m == MH - 1:
                        # last m-tile: relu on DVE, parallel with the
                        # ScalarE Exp (shortens the end-of-chunk chain)
                        nc.vector.tensor_scalar(
                            r[:], psum[:], b1_sb[:, m:m + 1], 0.0,
                            ALU.add, ALU.max,
                        )
                    else:
                        nc.scalar.activation(r[:], psum[:], AF.Relu,
                                             bias=b1_sb[:, m:m + 1])
                    h2 = h2p.tile([128, cw], F16, tag="h2", name="h2")
                    nc.vector.scalar_tensor_tensor(
                        h2[:], e[:], 1.0, r[:], ALU.subtract, ALU.min
                    )
                    a = accp.tile(
                        [128, cw], F16 if m >= MH - 2 else F32,
                        tag=f"acc{m % 4}",
                        name=f"acc_{n}_{m}_{c0}",
                    )
                    p = m % 2
                    if prev[p] is None:
                        nc.vector.tensor_scalar(
                            a[:], h2[:], w2_sb[:, m:m + 1], None, ALU.mult,
                        )
                    else:
                        nc.vector.scalar_tensor_tensor(
                            a[:], h2[:], w2_sb[:, m:m + 1], prev[p][:],
                            ALU.mult, ALU.add,
                        )
                    prev[p] = a
                if merge:
                    # merge the parity chains on DVE (hidden under the
                    # next chunk's matmuls) so the partition-reduce
                    # costs one PE slot instead of two (a GpSimd
                    # partition reduce was tried: far too slow)
                    sm = accp.tile([128, cw], F16, tag="accm",
                                   name=f"accm_{n}_{c0}")
                    nc.vector.tensor_tensor(sm[:], prev[0][:], prev[1][:],
                                            ALU.add)
                    out_ps = ops.tile([1, cw], F32, tag="ops",
                                      name=f"outps_{n}_{c0}")
                    nc.tensor.matmul(
                        out_ps[:], ones_sb[:], sm[:], start=True, stop=True,
                    )
                    z3_ap = out_ps[:]
                else:
                    # exposed final half: latency-optimal PE reduce,
                    # no serial DVE merge
                    out_ps = ops.tile([1, cw], F32, tag="ops",
                                      name=f"outps_{n}_{c0}")
                    nc.tensor.matmul(
                        out_ps[:], ones_sb[:], prev[0][:],
                        start=True, stop=False,
                    )
                    nc.tensor.matmul(
                        out_ps[:], ones_sb[:], prev[1][:],
                        start=False, stop=True,
                    )
                    z3_ap = out_ps[:]
                o = hpool.tile([1, cw], F32, tag="e", name="o")
                nc.scalar.activation(o[:], z3_ap, AF.Sigmoid,
                                     bias=b2_sb[:])
                t1 = redp.tile([1, cw], F32, tag="tred", name="t1")
                nc.scalar.activation(t1[:], o[:], AF.Abs,
                                     bias=c_ab[:], scale=c_sp[:])
                al = hpool.tile([1, cw], F32, tag="e", name="al")
                nc.scalar.activation(al[:], t1[:], AF.Relu, bias=c_rb[:])
                lo = n * CHUNK + c0
                nc.sync.dma_start(out_ext[0:1, lo:lo + cw], o[:])
                nc.sync.dma_start(alpha_ext[0:1, lo:lo + cw], al[:])

            # pipeline: L1 three chunks ahead of L2 (consume chunk n-3
            # BEFORE L1(n) writes into its ring slots - else deadlock)
            l1_chunk(0)
            l1_chunk(1)
            xt_tiles[3] = emit_xt(3)
            l1_chunk(2)
            for n in range(3, NCH):
                l2_chunk(n - 3)
                l1_chunk(n)
                if n + 1 < NCH:
                    xt_tiles[n + 1] = emit_xt(n + 1)
            l2_chunk(NCH - 3)
            l2_chunk(NCH - 2)
            # final chunk in two halves: the first half's epilogue hides
            # under the second half's matmuls
            # 384+128 split: the exposed last portion (its epilogue runs
            # after the very last matmul) is only 128 cols wide
            l2_chunk(NCH - 1, 0, 384, pop=False)
            l2_chunk(NCH - 1, 384, 128, merge=False)

    nc.compile()
    _NC_CACHE = nc
    return nc


LAST_RESULTS = None


def kernel(x, W0, b0, W1, b1, W2, b2):
    global LAST_RESULTS
    _install_profile_shim()
    from concourse.bass_utils import run_bass_kernel_spmd

    x = np.asarray(x, dtype=np.float32)
    W0 = np.ascontiguousarray(np.asarray(W0, dtype=np.float32))
    W1 = np.ascontiguousarray(np.asarray(W1, dtype=np.float32))
    W2 = np.asarray(W2, dtype=np.float32)
    b0 = np.asarray(b0, dtype=np.float32)
    b1 = np.asarray(b1, dtype=np.float32)
    b2 = np.asarray(b2, dtype=np.float32)

    nc = _build()

    # blocked DRAM layouts maximizing per-partition contiguity (see _build)
    w0b = np.ascontiguousarray(
        W0.astype(np.float16).reshape(KI, 128, W0NS, DH // W0NS)
        .transpose(2, 0, 1, 3).reshape(W0NS * KI * 128, DH // W0NS))
    w1b = np.ascontiguousarray(W1.astype(np.float16))
    w2r = np.ascontiguousarray(W2.reshape(KH, 128).T.astype(np.float32))
    b0r = np.ascontiguousarray(b0.reshape(MH, 128).T)
    b1r = np.ascontiguousarray(b1.reshape(MH, 128).T)
    b2r = b2.reshape(1, 1)

    in_maps = []
    for c in range(NCORES):
        shard = x[c * SHARD:(c + 1) * SHARD]
        xt = shard.T.astype(np.float16)          # [DIN, SHARD]
        xtb = np.ascontiguousarray(
            xt.reshape(KI, 128, NCH, CHUNK)
            .transpose(2, 1, 0, 3).reshape(NCH * 128, KI * CHUNK))
        xtkb = np.ascontiguousarray(
            xt.reshape(KI, 128, NCH, CHUNK)
            .transpose(2, 0, 1, 3)[0:3].reshape(3 * KI * 128, CHUNK))
        in_maps.append(
            {
                "xt": xtb,
                "xtkb": xtkb,
                "w0": w0b,
                "w1": w1b,
                "w2": w2r,
                "b0": b0r,
                "b1": b1r,
                "b2": b2r,
            }
        )

    # The first execution of a freshly-compiled NEFF intermittently hits a
    # transient device error (NRT_EXEC_UNIT_UNRECOVERABLE); a retry succeeds.
    import time as _time

    last_err = None
    for _attempt in range(3):
        try:
            res = run_bass_kernel_spmd(nc, in_maps, core_ids=list(range(NCORES)))
            break
        except Exception as e:  # noqa: BLE001 - retry transient device faults
            last_err = e
            _time.sleep(3.0)
    else:
        raise last_err
    LAST_RESULTS = res

    out = np.concatenate([res.results[c]["out"][0] for c in range(NCORES)])
    alpha = np.concatenate([res.results[c]["alpha"][0] for c in range(NCORES)])
    return out[:, None].astype(np.float32), alpha[:, None].astype(np.float32)


# revision 59
# speedup vs baseline: 1.0048x; 1.0048x over previous
"""AdapLSNet MLP kernel for 8 TRN2 NeuronCores (data-parallel).

reference:
    h  = elu(x @ W0 + b0)
    h  = elu(h @ W1 + b1)
    out = sigmoid(h @ W2 + b2)          # [B, 1]
    alpha = piecewise(out)               # a=0.1, b=0.2, c=0.8
    returns (out, alpha)

Strategy
- Shard batch (32768) across 8 cores (4096 rows each); replicate weights.
- Host pre-transposes each x shard to x^T so every layer's activations
  live in [feature(partitions), batch(free)] layout; no on-device
  transposes.  Per-chunk xt is stored as one [128, 4096] block (8KB
  per-partition contiguous runs -> large DMA packets).
- L1/L2 in fp16 (full PE rate, FWL weight loads, half the DMA/SBUF
  bytes; measured end-to-end rel err 1.0e-3 vs the 2e-2 gate).  fp8
  DoubleRow was evaluated and rejected: alpha has only ~8 nonzero tail
  entries, and fp8 noise on `out` gives alpha rel err 0.12-0.24.
- Single fused pass: W0 (fp16) and W1 (fp16) are SBUF-resident, so h1
  never leaves the chip.
- Startup is chip-HBM-contention-bound (~150-250 GB/s per core while
  all 8 cores pull their weights), so the DMA stream is ordered by
  first-use with minimal first-need bytes: b0/b1/b2/w2 first (b0 gates
  every L1 ScalarE activation - emitting it last deadlocked the psum
  ring for 23us and triggered a 48us HAM half-clock window); then the
  startup chunks xt0-2 in a k-block layout (eight [128,512] blocks per
  chunk, k order) interleaved pairwise with W0's first 128-col
  half-strip so L1 starts ~12us in and its k-loop PACES with block
  arrivals - any DMA wait appears as sub-3.4us micro-gaps that never
  re-throttle the HAM clock gate, instead of one big stall; then W0
  strip-major in 256-col strips (strip s of every slab before strip
  s+1, matching L1 m-tile consumption), then W1 as full slabs split
  into partition quarters (4KB runs).  Steady-state chunks use a wide
  [128, 4096] per-chunk block (8KB per-partition runs).  Transfers
  alternate the sync (HWDGE) and gpsimd (SWDGE) queue families.
- Software pipeline: L1 runs three batch-chunks ahead of L2 so the PE
  has L1 work while W1 streams in at startup.
- L3 (h2 @ W2, M=1) runs OFF the PE: per m-tile a single DVE
  scalar_tensor_tensor accumulates acc += w2[:,m] (x) h2 (per-partition
  scalar multiply) in two parity chains, and two ones-vector matmuls
  per chunk reduce the 128 partials -> z3 [1,512].  This frees 16 full
  512-col MM slots per chunk (~26us of PE time total) vs packed
  matmuls.
- elu(z) = min(exp(z) - 1, relu(z)): 2 ScalarE LUT ops reading PSUM with
  the bias fused + 1 fused VectorE (e-1) min r op; the last m-tile's
  relu runs on DVE in parallel with the ScalarE exp to shorten the
  end-of-chunk serial chain.
- alpha = relu(-0.5*out + 0.1) + relu(0.5*out - 0.4); the branches are
  mutually exclusive for out in [0,1], so it is computed as
  relu(|0.5*out - 0.25| - 0.15): 2 ScalarE ops, no DVE combine.
- The final chunk runs as two 256-col halves so most of its epilogue
  hides under the second half's matmuls.
- PE warmup matmuls keep the HAM clock gate released (2.4 GHz) across
  the initial DMA wait; NWARM is sized so warmup ends right as the
  first L1 inputs land (a >3.4us PE gap would re-throttle to 1.2 GHz).

Measured: 780.6us (baseline) -> ~696us on 8 axon trn2 cores,
rel err 1.1e-3 (gate 2e-2), ~94% of the fp16 PE roofline (656us).
"""

import numpy as np

BATCH = 32768
DIN = 1024
DH = 2048
NCORES = 8
SHARD = BATCH // NCORES          # 4096
CHUNK = 512
NCH = SHARD // CHUNK             # 8
KI = DIN // 128                  # 8
KH = DH // 128                   # 16
MH = DH // 128                   # 16
NH1S = 48                        # h1 slots (fp16 [128,512], 1KB each; 3 chunks)
NXTC = 4                         # xt chunk-tile ring ([128,4096] fp16, 8KB/part)
NWARM = 18                       # PE warmup matmuls (HAM un-throttle; sized
                                 # to end ~14us, inside the DMA-paced L1(0)
                                 # phase, so slow-DMA runs keep enough PE
                                 # activity in the HAM window to avoid a
                                 # half-clock blip; L1's first m-tile is
                                 # block-arrival-bound either way)
W0NS = 8                         # W0 strips per slab (256 cols)


def _install_profile_shim():
    """Allow trace=True under axon (exec_time_ns capture) if possible."""
    import sys
    import types

    try:
        import antenv

        if "antenv.axon_hooks" in sys.modules:
            return
        mod = types.ModuleType("antenv.axon_hooks")
        _hook = [None]
        mod.set_axon_ntff_profile_hook = lambda h: _hook.__setitem__(0, h)
        mod.get_axon_ntff_profile_hook = lambda: _hook[0]
        sys.modules["antenv.axon_hooks"] = mod
        antenv.axon_hooks = mod
        try:
            from trn_agent_boot.trn_boot import _ntff_profile_via_ctypes

            mod.set_axon_ntff_profile_hook(
                _ntff_profile_via_ctypes("/opt/axon/libaxon_pjrt.so")
            )
        except Exception:
            pass
    except Exception:
        pass


_NC_CACHE = None


def _build():
    global _NC_CACHE
    if _NC_CACHE is not None:
        return _NC_CACHE

    import concourse.mybir as mybir
    import concourse.tile as tile
    from concourse import bacc

    F32R = mybir.dt.float32r
    F32 = mybir.dt.float32
    F16 = mybir.dt.float16
    AF = mybir.ActivationFunctionType
    ALU = mybir.AluOpType

    nc = bacc.Bacc("TRN2", target_bir_lowering=False)

    # DMA packet size == per-PARTITION contiguous run length of the SBUF
    # destination; 1KB-run tiles capped the DMA engines at ~90-160 GB/s
    # and starved the startup pipeline.  So transfers below are
    # full-width row slices into wide tiles:
    # xt: chunk-tile blocks [128, KI*CHUNK] (partition p col k*512+c =
    #     xT[k*128+p, n*512+c]) -> 8KB/partition runs
    xt_ext = nc.declare_dram_parameter(
        "xt", [NCH * 128, KI * CHUNK], F16, isOutput=False)
    # first 3 chunks duplicated in k-block layout (row (n*KI+k)*128+p,
    # col c = xT[k*128+p, n*512+c]): the startup chunks stream as eight
    # [128,512] k-blocks each, so L1's k-loop paces with block arrivals
    # (sub-3.4us micro-gaps instead of one big HAM-re-throttling stall)
    xtkb_ext = nc.declare_dram_parameter(
        "xtkb", [3 * KI * 128, CHUNK], F16, isOutput=False)
    # w0: 2 half-width strips per slab: row (s*KI+k)*128+p, col c =
    #     W0[k*128+p, s*1024+c] -> 2KB/partition runs
    w0_ext = nc.declare_dram_parameter(
        "w0", [W0NS * KI * 128, DH // W0NS], F16, isOutput=False)
    # w1: original [DH, DH] layout; full slabs split by partition
    #     quarters -> 4KB/partition runs
    w1_ext = nc.declare_dram_parameter("w1", [DH, DH], F16, isOutput=False)
    w2_ext = nc.declare_dram_parameter("w2", [128, KH], F32, isOutput=False)
    b0_ext = nc.declare_dram_parameter("b0", [128, MH], F32, isOutput=False)
    b1_ext = nc.declare_dram_parameter("b1", [128, MH], F32, isOutput=False)
    b2_ext = nc.declare_dram_parameter("b2", [1, 1], F32, isOutput=False)
    out_ext = nc.declare_dram_parameter("out", [1, SHARD], F32, isOutput=True)
    alpha_ext = nc.declare_dram_parameter("alpha", [1, SHARD], F32, isOutput=True)

    with tile.TileContext(nc) as tc:
        with (
            tc.tile_pool(name="w0p", bufs=1) as w0p,
            tc.tile_pool(name="w1p", bufs=1) as w1p,
            tc.tile_pool(name="xtp", bufs=1) as xtp,
            tc.tile_pool(name="h1p", bufs=1) as h1p,
            tc.tile_pool(name="hpool", bufs=2) as hpool,
            tc.tile_pool(name="h2p", bufs=4) as h2p,
            tc.tile_pool(name="accp", bufs=1) as accp,
            tc.tile_pool(name="redp", bufs=2) as redp,
            tc.tile_pool(name="cst", bufs=1) as cst,
            tc.tile_pool(name="ps", bufs=6, space="PSUM") as ps,
            tc.tile_pool(name="ops", bufs=2, space="PSUM") as ops,
        ):
            w0_sb = [
                w0p.tile([128, DH], F16, tag=f"w0_{k}", name=f"w0_{k}")
                for k in range(KI)
            ]
            w1_sb = [
                w1p.tile([128, DH], F16, tag=f"w1_{k}", name=f"w1_{k}")
                for k in range(KH)
            ]

            def w0_lhsT(k, m):
                return w0_sb[k][:, m * 128:(m + 1) * 128]

            def w1_lhsT(k, m):
                return w1_sb[k][:, m * 128:(m + 1) * 128]

            def emit_xt(n, nsplit=8):
                """One [128, 4096] chunk tile, DMA'd as `nsplit`
                partition-range slices (keeps 8KB/partition packets,
                spreads across queues)."""
                t = xtp.tile([128, KI * CHUNK], F16, tag=f"xtc{n % NXTC}",
                             name=f"xt_{n}")
                rows = 128 // nsplit
                for j in range(nsplit):
                    eng = nc.sync if (j % 2 == 0) else nc.gpsimd
                    eng.dma_start(
                        t[j * rows:(j + 1) * rows, :],
                        xt_ext[n * 128 + j * rows:n * 128 + (j + 1) * rows, :],
                    )
                return t

            # --- small, first-use-critical tensors FIRST: b0 gates every
            # L1 ScalarE activation (and thence psum recycling) ---
            b0_sb = cst.tile([128, MH], F32, tag="b0", name="b0")
            nc.sync.dma_start(b0_sb[:], b0_ext[:])
            b1_sb = cst.tile([128, MH], F32, tag="b1", name="b1")
            nc.sync.dma_start(b1_sb[:], b1_ext[:])
            b2_sb = cst.tile([1, 1], F32, tag="b2", name="b2")
            nc.sync.dma_start(b2_sb[:], b2_ext[:])
            w2_sb = cst.tile([128, KH], F32, tag="w2", name="w2")
            nc.sync.dma_start(w2_sb[:], w2_ext[:])
            # alpha = relu(-0.5*o + 0.1) + relu(0.5*o - 0.4); the two
            # branches are mutually exclusive on o in [0,1], so
            # alpha = relu(|0.5*o - 0.25| - 0.15)  (2 ScalarE ops)
            c_ab = cst.tile([1, 1], F32, tag="c_ab", name="c_ab")
            c_rb = cst.tile([1, 1], F32, tag="c_rb", name="c_rb")
            c_sp = cst.tile([1, 1], F32, tag="c_sp", name="c_sp")
            nc.vector.memset(c_ab[:], -0.25)
            nc.vector.memset(c_rb[:], -0.15)
            nc.vector.memset(c_sp[:], 0.5)
            ones_sb = cst.tile([128, 1], F16, tag="ones", name="ones")
            nc.vector.memset(ones_sb[:], 1.0)

            def emit_xt_kb(n, fam=0):
                """Startup chunks: one [128, 4096] tile filled by eight
                [128, 512] k-block DMAs in k order (matches the L1
                k-loop's consumption order)."""
                t = xtp.tile([128, KI * CHUNK], F16, tag=f"xtc{n % NXTC}",
                             name=f"xt_{n}")
                for k in range(KI):
                    eng = nc.sync if ((k + fam) % 2 == 0) else nc.gpsimd
                    row = (n * KI + k) * 128
                    eng.dma_start(
                        t[:, k * CHUNK:(k + 1) * CHUNK],
                        xtkb_ext[row:row + 128, :],
                    )
                return t

            # --- startup stream, first-use-ordered and k-interleaved:
            # m0's k-step needs (xt0 block k, W0 slab k cols 0-127), so
            # emit those pairwise across the two queue families; L1 can
            # then start ~10us in and pace with arrivals. ---
            xt_tiles = {}
            t0 = xtp.tile([128, KI * CHUNK], F16, tag="xtc0", name="xt_0")
            xt_tiles[0] = t0
            for k in range(KI):
                nc.sync.dma_start(
                    t0[:, k * CHUNK:(k + 1) * CHUNK],
                    xtkb_ext[k * 128:(k + 1) * 128, :],
                )
                nc.gpsimd.dma_start(
                    w0_sb[k][:, 0:128],
                    w0_ext[k * 128:(k + 1) * 128, 0:128],
                )
            for k in range(KI):
                eng = nc.sync if (k % 2 == 1) else nc.gpsimd
                eng.dma_start(
                    w0_sb[k][:, 128:256],
                    w0_ext[k * 128:(k + 1) * 128, 128:256],
                )
            W0S = DH // W0NS         # 256-col strips, strip-major
            for s in range(1, W0NS):
                for k in range(KI):
                    eng = nc.sync if ((s + k) % 2 == 0) else nc.gpsimd
                    row = (s * KI + k) * 128
                    eng.dma_start(
                        w0_sb[k][:, s * W0S:(s + 1) * W0S],
                        w0_ext[row:row + 128, :],
                    )

            xt_tiles[1] = emit_xt_kb(1, fam=0)
            xt_tiles[2] = emit_xt_kb(2, fam=1)

            # --- W1: full slabs as 4 partition quarters [32, 2048]
            # (source rows 128k+32q..+32 are contiguous 128KB) ---
            for k in range(KH):
                for q in range(4):
                    eng = nc.sync if ((k + q) % 2 == 0) else nc.gpsimd
                    r0 = 128 * k + 32 * q
                    eng.dma_start(w1_sb[k][32 * q:32 * q + 32, :],
                                  w1_ext[r0:r0 + 32, :])

            # PE warmup: dependency-free matmuls on a memset tile keep the
            # PE busy during the initial DMA wait so the HAM clock gate is
            # already released (2.4 GHz) when real matmuls start.
            wu = hpool.tile([128, CHUNK], F16, tag="e", name="wu")
            nc.vector.memset(wu[:], 0.0)
            for i in range(NWARM):
                wps = ps.tile([128, CHUNK], F32, tag="ps", name=f"wups_{i}")
                nc.tensor.matmul(
                    wps[:], wu[:, 0:128], wu[:], start=True, stop=True,
                )

            h1_tiles = {}

            def l1_chunk(n):
                """L1: h1(n) = elu(W0.T @ xT(n) + b0), kept in SBUF."""
                xt_sb = xt_tiles.pop(n)
                h1base = (MH * n) % NH1S
                tiles = []
                for m in range(MH):
                    psum = ps.tile([128, CHUNK], F32, tag="ps",
                                   name=f"psA_{n}_{m}")
                    for k in range(KI):
                        nc.tensor.matmul(
                            psum[:], w0_lhsT(k, m),
                            xt_sb[:, k * CHUNK:(k + 1) * CHUNK],
                            start=(k == 0), stop=(k == KI - 1),
                        )
                    e = hpool.tile([128, CHUNK], F32, tag="e", name="e")
                    r = hpool.tile([128, CHUNK], F32, tag="r", name="r")
                    nc.scalar.activation(e[:], psum[:], AF.Exp,
                                         bias=b0_sb[:, m:m + 1])
                    nc.scalar.activation(r[:], psum[:], AF.Relu,
                                         bias=b0_sb[:, m:m + 1])
                    h1 = h1p.tile(
                        [128, CHUNK], F16, tag=f"h{(h1base + m) % NH1S}",
                        name=f"h1_{n}_{m}",
                    )
                    nc.vector.scalar_tensor_tensor(
                        h1[:], e[:], 1.0, r[:], ALU.subtract, ALU.min
                    )
                    tiles.append(h1)
                h1_tiles[n] = tiles

            def l2_chunk(n, c0=0, cw=CHUNK, pop=True, merge=True):
                """L2 + L3 + sigmoid + alpha for cols [c0, c0+cw) of
                chunk n.

                L3 runs off the PE: a DVE scalar_tensor_tensor chain
                accumulates acc += w2[:,m] (x) h2 per m-tile, then one
                ones-vector matmul reduces partitions -> z3 [1,cw].
                The final chunk runs as two halves so most of its
                epilogue hides under the second half's matmuls.
                """
                h1_sb = h1_tiles[n]
                if pop:
                    del h1_tiles[n]
                prev = [None, None]       # even / odd m accumulation chains
                for m in range(MH):
                    psum = ps.tile([128, cw], F32, tag="ps",
                                   name=f"psB_{n}_{m}_{c0}")
                    for k in range(KH):
                        nc.tensor.matmul(
                            psum[:], w1_lhsT(k, m),
                            h1_sb[k][:, c0:c0 + cw],
                            start=(k == 0), stop=(k == KH - 1),
                        )
                    e = hpool.tile([128, cw], F32, tag="e", name="e")
                    r = hpool.tile([128, cw], F32, tag="r", name="r")
                    nc.scalar.activation(e[:], psum[:], AF.Exp,
                                         bias=b1_sb[:, m:m + 1])
                    if m == MH - 1:
                        # last m-tile: relu on DVE, parallel with the
                        # ScalarE Exp (shortens the end-of-chunk chain)
                        nc.vector.tensor_scalar(
                            r[:], psum[:], b1_sb[:, m:m + 1], 0.0,
                            ALU.add, ALU.max,
                        )
                    else:
                        nc.scalar.activation(r[:], psum[:], AF.Relu,
                                             bias=b1_sb[:, m:m + 1])
                    h2 = h2p.tile([128, cw], F16, tag="h2", name="h2")
                    nc.vector.scalar_tensor_tensor(
                        h2[:], e[:], 1.0, r[:], ALU.subtract, ALU.min
                    )
                    a = accp.tile(
                        [128, cw], F16 if m >= MH - 2 else F32,
                        tag=f"acc{m % 4}",
                        name=f"acc_{n}_{m}_{c0}",
                    )
                    p = m % 2
                    if prev[p] is None:
                        nc.vector.tensor_scalar(
                            a[:], h2[:], w2_sb[:, m:m + 1], None, ALU.mult,
                        )
                    else:
                        nc.vector.scalar_tensor_tensor(
                            a[:], h2[:], w2_sb[:, m:m + 1], prev[p][:],
                            ALU.mult, ALU.add,
                        )
                    prev[p] = a
                if merge:
                    # merge the parity chains on DVE (hidden under the
                    # next chunk's matmuls) so the partition-reduce
                    # costs one PE slot instead of two (a GpSimd
                    # partition reduce was tried: far too slow)
                    sm = accp.tile([128, cw], F16, tag="accm",
                                   name=f"accm_{n}_{c0}")
                    nc.vector.tensor_tensor(sm[:], prev[0][:], prev[1][:],
                                            ALU.add)
                    out_ps = ops.tile([1, cw], F32, tag="ops",
                                      name=f"outps_{n}_{c0}")
                    nc.tensor.matmul(
                        out_ps[:], ones_sb[:], sm[:], start=True, stop=True,
                    )
                    z3_ap = out_ps[:]
                else:
                    # exposed final half: latency-optimal PE reduce,
                    # no serial DVE merge
                    out_ps = ops.tile([1, cw], F32, tag="ops",
                                      name=f"outps_{n}_{c0}")
                    nc.tensor.matmul(
                        out_ps[:], ones_sb[:], prev[0][:],
                        start=True, stop=False,
                    )
                    nc.tensor.matmul(
                        out_ps[:], ones_sb[:], prev[1][:],
                        start=False, stop=True,
                    )
                    z3_ap = out_ps[:]
                o = hpool.tile([1, cw], F32, tag="e", name="o")
                nc.scalar.activation(o[:], z3_ap, AF.Sigmoid,
                                     bias=b2_sb[:])
                t1 = redp.tile([1, cw], F32, tag="tred", name="t1")
                nc.scalar.activation(t1[:], o[:], AF.Abs,
                                     bias=c_ab[:], scale=c_sp[:])
                al = hpool.tile([1, cw], F32, tag="e", name="al")
                nc.scalar.activation(al[:], t1[:], AF.Relu, bias=c_rb[:])
                lo = n * CHUNK + c0
                nc.sync.dma_start(out_ext[0:1, lo:lo + cw], o[:])
                nc.sync.dma_start(alpha_ext[0:1, lo:lo + cw], al[:])

            # pipeline: L1 three chunks ahead of L2 (consume chunk n-3
            # BEFORE L1(n) writes into its ring slots - else deadlock)
            l1_chunk(0)
            l1_chunk(1)
            xt_tiles[3] = emit_xt(3)
            l1_chunk(2)
            for n in range(3, NCH):
                l2_chunk(n - 3)
                l1_chunk(n)
                if n + 1 < NCH:
                    xt_tiles[n + 1] = emit_xt(n + 1)
            l2_chunk(NCH - 3)
            l2_chunk(NCH - 2)
            # final chunk in two halves: the first half's epilogue hides
            # under the second half's matmuls
            l2_chunk(NCH - 1, 0, CHUNK // 2, pop=False)
            l2_chunk(NCH - 1, CHUNK // 2, CHUNK // 2, merge=False)

    nc.compile()
    _NC_CACHE = nc
    return nc


LAST_RESULTS = None


def kernel(x, W0, b0, W1, b1, W2, b2):
    global LAST_RESULTS
    _install_profile_shim()
    from concourse.bass_utils import run_bass_kernel_spmd

    x = np.asarray(x, dtype=np.float32)
    W0 = np.ascontiguousarray(np.asarray(W0, dtype=np.float32))
    W1 = np.ascontiguousarray(np.asarray(W1, dtype=np.float32))
    W2 = np.asarray(W2, dtype=np.float32)
    b0 = np.asarray(b0, dtype=np.float32)
    b1 = np.asarray(b1, dtype=np.float32)
    b2 = np.asarray(b2, dtype=np.float32)

    nc = _build()

    # blocked DRAM layouts maximizing per-partition contiguity (see _build)
    w0b = np.ascontiguousarray(
        W0.astype(np.float16).reshape(KI, 128, W0NS, DH // W0NS)
        .transpose(2, 0, 1, 3).reshape(W0NS * KI * 128, DH // W0NS))
    w1b = np.ascontiguousarray(W1.astype(np.float16))
    w2r = np.ascontiguousarray(W2.reshape(KH, 128).T.astype(np.float32))
    b0r = np.ascontiguousarray(b0.reshape(MH, 128).T)
    b1r = np.ascontiguousarray(b1.reshape(MH, 128).T)
    b2r = b2.reshape(1, 1)

    in_maps = []
    for c in range(NCORES):
        shard = x[c * SHARD:(c + 1) * SHARD]
        xt = shard.T.astype(np.float16)          # [DIN, SHARD]
        xtb = np.ascontiguousarray(
            xt.reshape(KI, 128, NCH, CHUNK)
            .transpose(2, 1, 0, 3).reshape(NCH * 128, KI * CHUNK))
        xtkb = np.ascontiguousarray(
            xt.reshape(KI, 128, NCH, CHUNK)
            .transpose(2, 0, 1, 3)[0:3].reshape(3 * KI * 128, CHUNK))
        in_maps.append(
            {
                "xt": xtb,
                "xtkb": xtkb,
                "w0": w0b,
                "w1": w1b,
                "w2": w2r,
                "b0": b0r,
                "b1": b1r,
                "b2": b2r,
            }
        )

    # The first execution of a freshly-compiled NEFF intermittently hits a
    # transient device error (NRT_EXEC_UNIT_UNRECOVERABLE); a retry succeeds.
    import time as _time

    last_err = None
    for _attempt in range(3):
        try:
            res = run_bass_kernel_spmd(nc, in_maps, core_ids=list(range(NCORES)))
            break
        except Exception as e:  # noqa: BLE001 - retry transient device faults
            last_err = e
            _time.sleep(3.0)
    else:
        raise last_err
    LAST_RESULTS = res

    out = np.concatenate([res.results[c]["out"][0] for c in range(NCORES)])
    alpha = np.concatenate([res.results[c]["alpha"][0] for c in range(NCORES)])
    return out[:, None].astype(np.float32), alpha[:, None].astype(np.float32)


# revision 60
# speedup vs baseline: 1.0064x; 1.0016x over previous
"""AdapLSNet MLP kernel for 8 TRN2 NeuronCores (data-parallel).

reference:
    h  = elu(x @ W0 + b0)
    h  = elu(h @ W1 + b1)
    out = sigmoid(h @ W2 + b2)          # [B, 1]
    alpha = piecewise(out)               # a=0.1, b=0.2, c=0.8
    returns (out, alpha)

Strategy
- Shard batch (32768) across 8 cores (4096 rows each); replicate weights.
- Host pre-transposes each x shard to x^T so every layer's activations
  live in [feature(partitions), batch(free)] layout; no on-device
  transposes.  Per-chunk xt is stored as one [128, 4096] block (8KB
  per-partition contiguous runs -> large DMA packets).
- L1/L2 in fp16 (full PE rate, FWL weight loads, half the DMA/SBUF
  bytes; measured end-to-end rel err 1.0e-3 vs the 2e-2 gate).  fp8
  DoubleRow was evaluated and rejected: alpha has only ~8 nonzero tail
  entries, and fp8 noise on `out` gives alpha rel err 0.12-0.24.
- Single fused pass: W0 (fp16) and W1 (fp16) are SBUF-resident, so h1
  never leaves the chip.
- Startup is chip-HBM-contention-bound (~150-250 GB/s per core while
  all 8 cores pull their weights), so the DMA stream is ordered by
  first-use with minimal first-need bytes: b0/b1/b2/w2 first (b0 gates
  every L1 ScalarE activation - emitting it last deadlocked the psum
  ring for 23us and triggered a 48us HAM half-clock window); then the
  startup chunks xt0-2 in a k-block layout (eight [128,512] blocks per
  chunk, k order) interleaved pairwise with W0's first 128-col
  half-strip so L1 starts ~12us in and its k-loop PACES with block
  arrivals - any DMA wait appears as sub-3.4us micro-gaps that never
  re-throttle the HAM clock gate, instead of one big stall; then W0
  strip-major in 256-col strips (strip s of every slab before strip
  s+1, matching L1 m-tile consumption), then W1 as full slabs split
  into partition quarters (4KB runs).  Steady-state chunks use a wide
  [128, 4096] per-chunk block (8KB per-partition runs).  Transfers
  alternate the sync (HWDGE) and gpsimd (SWDGE) queue families.
- Software pipeline: L1 runs three batch-chunks ahead of L2 so the PE
  has L1 work while W1 streams in at startup.
- L3 (h2 @ W2, M=1) runs OFF the PE: per m-tile a single DVE
  scalar_tensor_tensor accumulates acc += w2[:,m] (x) h2 (per-partition
  scalar multiply) in two parity chains, and two ones-vector matmuls
  per chunk reduce the 128 partials -> z3 [1,512].  This frees 16 full
  512-col MM slots per chunk (~26us of PE time total) vs packed
  matmuls.
- elu(z) = min(exp(z) - 1, relu(z)): 2 ScalarE LUT ops reading PSUM with
  the bias fused + 1 fused VectorE (e-1) min r op; the last m-tile's
  relu runs on DVE in parallel with the ScalarE exp to shorten the
  end-of-chunk serial chain.
- alpha = relu(-0.5*out + 0.1) + relu(0.5*out - 0.4); the branches are
  mutually exclusive for out in [0,1], so it is computed as
  relu(|0.5*out - 0.25| - 0.15): 2 ScalarE ops, no DVE combine.
- The final chunk runs as two 256-col halves so most of its epilogue
  hides under the second half's matmuls.
- PE warmup matmuls keep the HAM clock gate released (2.4 GHz) across
  the initial DMA wait; NWARM is sized so warmup ends right as the
  first L1 inputs land (a >3.4us PE gap would re-throttle to 1.2 GHz).

Measured: 780.6us (baseline) -> ~696us on 8 axon trn2 cores,
rel err 1.1e-3 (gate 2e-2), ~94% of the fp16 PE roofline (656us).
"""

import numpy as np

BATCH = 32768
DIN = 1024
DH = 2048
NCORES = 8
SHARD = BATCH // NCORES          # 4096
CHUNK = 512
NCH = SHARD // CHUNK             # 8
KI = DIN // 128                  # 8
KH = DH // 128                   # 16
MH = DH // 128                   # 16
NH1S = 48                        # h1 slots (fp16 [128,512], 1KB each; 3 chunks)
NXTC = 4                         # xt chunk-tile ring ([128,4096] fp16, 8KB/part)
NWARM = 26                       # PE warmup matmuls (HAM un-throttle; sized
                                 # to end ~14us, inside the DMA-paced L1(0)
                                 # phase, so slow-DMA runs keep enough PE
                                 # activity in the HAM window to avoid a
                                 # half-clock blip; L1's first m-tile is
                                 # block-arrival-bound either way)
W0NS = 8                         # W0 strips per slab (256 cols)


def _install_profile_shim():
    """Allow trace=True under axon (exec_time_ns capture) if possible."""
    import sys
    import types

    try:
        import antenv

        if "antenv.axon_hooks" in sys.modules:
            return
        mod = types.ModuleType("antenv.axon_hooks")
        _hook = [None]
        mod.set_axon_ntff_profile_hook = lambda h: _hook.__setitem__(0, h)
        mod.get_axon_ntff_profile_hook = lambda: _hook[0]
        sys.modules["antenv.axon_hooks"] = mod
        antenv.axon_hooks = mod
        try:
            from trn_agent_boot.trn_boot import _ntff_profile_via_ctypes

            mod.set_axon_ntff_profile_hook(
                _ntff_profile_via_ctypes("/opt/axon/libaxon_pjrt.so")
            )
        except Exception:
            pass
    except Exception:
        pass


_NC_CACHE = None


def _build():
    global _NC_CACHE
    if _NC_CACHE is not None:
        return _NC_CACHE

    import concourse.mybir as mybir
    import concourse.tile as tile
    from concourse import bacc

    F32R = mybir.dt.float32r
    F32 = mybir.dt.float32
    F16 = mybir.dt.float16
    AF = mybir.ActivationFunctionType
    ALU = mybir.AluOpType

    nc = bacc.Bacc("TRN2", target_bir_lowering=False)

    # DMA packet size == per-PARTITION contiguous run length of the SBUF
    # destination; 1KB-run tiles capped the DMA engines at ~90-160 GB/s
    # and starved the startup pipeline.  So transfers below are
    # full-width row slices into wide tiles:
    # xt: chunk-tile blocks [128, KI*CHUNK] (partition p col k*512+c =
    #     xT[k*128+p, n*512+c]) -> 8KB/partition runs
    xt_ext = nc.declare_dram_parameter(
        "xt", [NCH * 128, KI * CHUNK], F16, isOutput=False)
    # first 3 chunks duplicated in k-block layout (row (n*KI+k)*128+p,
    # col c = xT[k*128+p, n*512+c]): the startup chunks stream as eight
    # [128,512] k-blocks each, so L1's k-loop paces with block arrivals
    # (sub-3.4us micro-gaps instead of one big HAM-re-throttling stall)
    xtkb_ext = nc.declare_dram_parameter(
        "xtkb", [3 * KI * 128, CHUNK], F16, isOutput=False)
    # w0: 2 half-width strips per slab: row (s*KI+k)*128+p, col c =
    #     W0[k*128+p, s*1024+c] -> 2KB/partition runs
    w0_ext = nc.declare_dram_parameter(
        "w0", [W0NS * KI * 128, DH // W0NS], F16, isOutput=False)
    # w1: original [DH, DH] layout; full slabs split by partition
    #     quarters -> 4KB/partition runs
    w1_ext = nc.declare_dram_parameter("w1", [DH, DH], F16, isOutput=False)
    w2_ext = nc.declare_dram_parameter("w2", [128, KH], F32, isOutput=False)
    b0_ext = nc.declare_dram_parameter("b0", [128, MH], F32, isOutput=False)
    b1_ext = nc.declare_dram_parameter("b1", [128, MH], F32, isOutput=False)
    b2_ext = nc.declare_dram_parameter("b2", [1, 1], F32, isOutput=False)
    out_ext = nc.declare_dram_parameter("out", [1, SHARD], F32, isOutput=True)
    alpha_ext = nc.declare_dram_parameter("alpha", [1, SHARD], F32, isOutput=True)

    with tile.TileContext(nc) as tc:
        with (
            tc.tile_pool(name="w0p", bufs=1) as w0p,
            tc.tile_pool(name="w1p", bufs=1) as w1p,
            tc.tile_pool(name="xtp", bufs=1) as xtp,
            tc.tile_pool(name="h1p", bufs=1) as h1p,
            tc.tile_pool(name="hpool", bufs=2) as hpool,
            tc.tile_pool(name="h2p", bufs=4) as h2p,
            tc.tile_pool(name="accp", bufs=1) as accp,
            tc.tile_pool(name="redp", bufs=2) as redp,
            tc.tile_pool(name="cst", bufs=1) as cst,
            tc.tile_pool(name="ps", bufs=6, space="PSUM") as ps,
            tc.tile_pool(name="ops", bufs=2, space="PSUM") as ops,
        ):
            w0_sb = [
                w0p.tile([128, DH], F16, tag=f"w0_{k}", name=f"w0_{k}")
                for k in range(KI)
            ]
            w1_sb = [
                w1p.tile([128, DH], F16, tag=f"w1_{k}", name=f"w1_{k}")
                for k in range(KH)
            ]

            def w0_lhsT(k, m):
                return w0_sb[k][:, m * 128:(m + 1) * 128]

            def w1_lhsT(k, m):
                return w1_sb[k][:, m * 128:(m + 1) * 128]

            def emit_xt(n, nsplit=8):
                """One [128, 4096] chunk tile, DMA'd as `nsplit`
                partition-range slices (keeps 8KB/partition packets,
                spreads across queues)."""
                t = xtp.tile([128, KI * CHUNK], F16, tag=f"xtc{n % NXTC}",
                             name=f"xt_{n}")
                rows = 128 // nsplit
                for j in range(nsplit):
                    eng = nc.sync if (j % 2 == 0) else nc.gpsimd
                    eng.dma_start(
                        t[j * rows:(j + 1) * rows, :],
                        xt_ext[n * 128 + j * rows:n * 128 + (j + 1) * rows, :],
                    )
                return t

            # --- small, first-use-critical tensors FIRST: b0 gates every
            # L1 ScalarE activation (and thence psum recycling) ---
            b0_sb = cst.tile([128, MH], F32, tag="b0", name="b0")
            nc.sync.dma_start(b0_sb[:], b0_ext[:])
            b1_sb = cst.tile([128, MH], F32, tag="b1", name="b1")
            nc.sync.dma_start(b1_sb[:], b1_ext[:])
            b2_sb = cst.tile([1, 1], F32, tag="b2", name="b2")
            nc.sync.dma_start(b2_sb[:], b2_ext[:])
            w2_sb = cst.tile([128, KH], F32, tag="w2", name="w2")
            nc.sync.dma_start(w2_sb[:], w2_ext[:])
            # alpha = relu(-0.5*o + 0.1) + relu(0.5*o - 0.4); the two
            # branches are mutually exclusive on o in [0,1], so
            # alpha = relu(|0.5*o - 0.25| - 0.15)  (2 ScalarE ops)
            c_ab = cst.tile([1, 1], F32, tag="c_ab", name="c_ab")
            c_rb = cst.tile([1, 1], F32, tag="c_rb", name="c_rb")
            c_sp = cst.tile([1, 1], F32, tag="c_sp", name="c_sp")
            nc.vector.memset(c_ab[:], -0.25)
            nc.vector.memset(c_rb[:], -0.15)
            nc.vector.memset(c_sp[:], 0.5)
            ones_sb = cst.tile([128, 1], F16, tag="ones", name="ones")
            nc.vector.memset(ones_sb[:], 1.0)

            def emit_xt_kb(n, fam=0):
                """Startup chunks: one [128, 4096] tile filled by eight
                [128, 512] k-block DMAs in k order (matches the L1
                k-loop's consumption order)."""
                t = xtp.tile([128, KI * CHUNK], F16, tag=f"xtc{n % NXTC}",
                             name=f"xt_{n}")
                for k in range(KI):
                    eng = nc.sync if ((k + fam) % 2 == 0) else nc.gpsimd
                    row = (n * KI + k) * 128
                    eng.dma_start(
                        t[:, k * CHUNK:(k + 1) * CHUNK],
                        xtkb_ext[row:row + 128, :],
                    )
                return t

            # --- startup stream, first-use-ordered and k-interleaved:
            # m0's k-step needs (xt0 block k, W0 slab k cols 0-127), so
            # emit those pairwise across the two queue families; L1 can
            # then start ~10us in and pace with arrivals. ---
            xt_tiles = {}
            t0 = xtp.tile([128, KI * CHUNK], F16, tag="xtc0", name="xt_0")
            xt_tiles[0] = t0
            for k in range(KI):
                nc.sync.dma_start(
                    t0[:, k * CHUNK:(k + 1) * CHUNK],
                    xtkb_ext[k * 128:(k + 1) * 128, :],
                )
                nc.gpsimd.dma_start(
                    w0_sb[k][:, 0:128],
                    w0_ext[k * 128:(k + 1) * 128, 0:128],
                )
            for k in range(KI):
                eng = nc.sync if (k % 2 == 1) else nc.gpsimd
                eng.dma_start(
                    w0_sb[k][:, 128:256],
                    w0_ext[k * 128:(k + 1) * 128, 128:256],
                )
            W0S = DH // W0NS         # 256-col strips, strip-major
            for s in range(1, W0NS):
                for k in range(KI):
                    eng = nc.sync if ((s + k) % 2 == 0) else nc.gpsimd
                    row = (s * KI + k) * 128
                    eng.dma_start(
                        w0_sb[k][:, s * W0S:(s + 1) * W0S],
                        w0_ext[row:row + 128, :],
                    )

            xt_tiles[1] = emit_xt_kb(1, fam=0)
            xt_tiles[2] = emit_xt_kb(2, fam=1)

            # --- W1: full slabs as 4 partition quarters [32, 2048]
            # (source rows 128k+32q..+32 are contiguous 128KB) ---
            for k in range(KH):
                for q in range(4):
                    eng = nc.sync if ((k + q) % 2 == 0) else nc.gpsimd
                    r0 = 128 * k + 32 * q
                    eng.dma_start(w1_sb[k][32 * q:32 * q + 32, :],
                                  w1_ext[r0:r0 + 32, :])

            # PE warmup: dependency-free matmuls on a memset tile keep the
            # PE busy during the initial DMA wait so the HAM clock gate is
            # already released (2.4 GHz) when real matmuls start.
            wu = hpool.tile([128, CHUNK], F16, tag="e", name="wu")
            nc.vector.memset(wu[:], 0.0)
            for i in range(NWARM):
                wps = ps.tile([128, CHUNK], F32, tag="ps", name=f"wups_{i}")
                nc.tensor.matmul(
                    wps[:], wu[:, 0:128], wu[:], start=True, stop=True,
                )

            h1_tiles = {}

            def l1_chunk(n):
                """L1: h1(n) = elu(W0.T @ xT(n) + b0), kept in SBUF."""
                xt_sb = xt_tiles.pop(n)
                h1base = (MH * n) % NH1S
                tiles = []
                for m in range(MH):
                    psum = ps.tile([128, CHUNK], F32, tag="ps",
                                   name=f"psA_{n}_{m}")
                    for k in range(KI):
                        nc.tensor.matmul(
                            psum[:], w0_lhsT(k, m),
                            xt_sb[:, k * CHUNK:(k + 1) * CHUNK],
                            start=(k == 0), stop=(k == KI - 1),
                        )
                    e = hpool.tile([128, CHUNK], F32, tag="e", name="e")
                    r = hpool.tile([128, CHUNK], F32, tag="r", name="r")
                    nc.scalar.activation(e[:], psum[:], AF.Exp,
                                         bias=b0_sb[:, m:m + 1])
                    nc.scalar.activation(r[:], psum[:], AF.Relu,
                                         bias=b0_sb[:, m:m + 1])
                    h1 = h1p.tile(
                        [128, CHUNK], F16, tag=f"h{(h1base + m) % NH1S}",
                        name=f"h1_{n}_{m}",
                    )
                    nc.vector.scalar_tensor_tensor(
                        h1[:], e[:], 1.0, r[:], ALU.subtract, ALU.min
                    )
                    tiles.append(h1)
                h1_tiles[n] = tiles

            def l2_chunk(n, c0=0, cw=CHUNK, pop=True, merge=True):
                """L2 + L3 + sigmoid + alpha for cols [c0, c0+cw) of
                chunk n.

                L3 runs off the PE: a DVE scalar_tensor_tensor chain
                accumulates acc += w2[:,m] (x) h2 per m-tile, then one
                ones-vector matmul reduces partitions -> z3 [1,cw].
                The final chunk runs as two halves so most of its
                epilogue hides under the second half's matmuls.
                """
                h1_sb = h1_tiles[n]
                if pop:
                    del h1_tiles[n]
                prev = [None, None]       # even / odd m accumulation chains
                for m in range(MH):
                    psum = ps.tile([128, cw], F32, tag="ps",
                                   name=f"psB_{n}_{m}_{c0}")
                    for k in range(KH):
                        nc.tensor.matmul(
                            psum[:], w1_lhsT(k, m),
                            h1_sb[k][:, c0:c0 + cw],
                            start=(k == 0), stop=(k == KH - 1),
                        )
                    e = hpool.tile([128, cw], F32, tag="e", name="e")
                    r = hpool.tile([128, cw], F32, tag="r", name="r")
                    nc.scalar.activation(e[:], psum[:], AF.Exp,
                                         bias=b1_sb[:, m:m + 1])
                    if m == MH - 1:
                        # last m-tile: relu on DVE, parallel with the
                        # ScalarE Exp (shortens the end-of-chunk chain)
                        nc.vector.tensor_scalar(
                            r[:], psum[:], b1_sb[:, m:m + 1], 0.0,
                            ALU.add, ALU.max,
                        )
                    else:
                        nc.scalar.activation(r[:], psum[:], AF.Relu,
                                             bias=b1_sb[:, m:m + 1])
                    h2 = h2p.tile([128, cw], F16, tag="h2", name="h2")
                    nc.vector.scalar_tensor_tensor(
                        h2[:], e[:], 1.0, r[:], ALU.subtract, ALU.min
                    )
                    a = accp.tile(
                        [128, cw], F16 if m >= MH - 2 else F32,
                        tag=f"acc{m % 4}",
                        name=f"acc_{n}_{m}_{c0}",
                    )
                    p = m % 2
                    if prev[p] is None:
                        nc.vector.tensor_scalar(
                            a[:], h2[:], w2_sb[:, m:m + 1], None, ALU.mult,
                        )
                    else:
                        nc.vector.scalar_tensor_tensor(
                            a[:], h2[:], w2_sb[:, m:m + 1], prev[p][:],
                            ALU.mult, ALU.add,
                        )
                    prev[p] = a
                if merge:
                    # merge the parity chains on DVE (hidden under the
                    # next chunk's matmuls) so the partition-reduce
                    # costs one PE slot instead of two (a GpSimd
                    # partition reduce was tried: far too slow)
                    sm = accp.tile([128, cw], F16, tag="accm",
                                   name=f"accm_{n}_{c0}")
                    nc.vector.tensor_tensor(sm[:], prev[0][:], prev[1][:],
                                            ALU.add)
                    out_ps = ops.tile([1, cw], F32, tag="ops",
                                      name=f"outps_{n}_{c0}")
                    nc.tensor.matmul(
                        out_ps[:], ones_sb[:], sm[:], start=True, stop=True,
                    )
                    z3_ap = out_ps[:]
                else:
                    # exposed final half: latency-optimal PE reduce,
                    # no serial DVE merge
                    out_ps = ops.tile([1, cw], F32, tag="ops",
                                      name=f"outps_{n}_{c0}")
                    nc.tensor.matmul(
                        out_ps[:], ones_sb[:], prev[0][:],
                        start=True, stop=False,
                    )
                    nc.tensor.matmul(
                        out_ps[:], ones_sb[:], prev[1][:],
                        start=False, stop=True,
                    )
                    z3_ap = out_ps[:]
                o = hpool.tile([1, cw], F32, tag="e", name="o")
                nc.scalar.activation(o[:], z3_ap, AF.Sigmoid,
                                     bias=b2_sb[:])
                t1 = redp.tile([1, cw], F32, tag="tred", name="t1")
                nc.scalar.activation(t1[:], o[:], AF.Abs,
                                     bias=c_ab[:], scale=c_sp[:])
                al = hpool.tile([1, cw], F32, tag="e", name="al")
                nc.scalar.activation(al[:], t1[:], AF.Relu, bias=c_rb[:])
                lo = n * CHUNK + c0
                nc.sync.dma_start(out_ext[0:1, lo:lo + cw], o[:])
                nc.sync.dma_start(alpha_ext[0:1, lo:lo + cw], al[:])

            # pipeline: L1 three chunks ahead of L2 (consume chunk n-3
            # BEFORE L1(n) writes into its ring slots - else deadlock)
            l1_chunk(0)
            l1_chunk(1)
            xt_tiles[3] = emit_xt(3)
            l1_chunk(2)
            for n in range(3, NCH):
                l2_chunk(n - 3)
                l1_chunk(n)
                if n + 1 < NCH:
                    xt_tiles[n + 1] = emit_xt(n + 1)
            l2_chunk(NCH - 3)
            l2_chunk(NCH - 2)
            # final chunk in two halves: the first half's epilogue hides
            # under the second half's matmuls
            l2_chunk(NCH - 1, 0, CHUNK // 2, pop=False)
            l2_chunk(NCH - 1, CHUNK // 2, CHUNK // 2, merge=False)

    nc.compile()
    _NC_CACHE = nc
    return nc


LAST_RESULTS = None


def kernel(x, W0, b0, W1, b1, W2, b2):
    global LAST_RESULTS
    _install_profile_shim()
    from concourse.bass_utils import run_bass_kernel_spmd

    x = np.asarray(x, dtype=np.float32)
    W0 = np.ascontiguousarray(np.asarray(W0, dtype=np.float32))
    W1 = np.ascontiguousarray(np.asarray(W1, dtype=np.float32))
    W2 = np.asarray(W2, dtype=np.float32)
    b0 = np.asarray(b0, dtype=np.float32)
    b1 = np.asarray(b1, dtype=np.float32)
    b2 = np.asarray(b2, dtype=np.float32)

    nc = _build()

    # blocked DRAM layouts maximizing per-partition contiguity (see _build)
    w0b = np.ascontiguousarray(
        W0.astype(np.float16).reshape(KI, 128, W0NS, DH // W0NS)
        .transpose(2, 0, 1, 3).reshape(W0NS * KI * 128, DH // W0NS))
    w1b = np.ascontiguousarray(W1.astype(np.float16))
    w2r = np.ascontiguousarray(W2.reshape(KH, 128).T.astype(np.float32))
    b0r = np.ascontiguousarray(b0.reshape(MH, 128).T)
    b1r = np.ascontiguousarray(b1.reshape(MH, 128).T)
    b2r = b2.reshape(1, 1)

    in_maps = []
    for c in range(NCORES):
        shard = x[c * SHARD:(c + 1) * SHARD]
        xt = shard.T.astype(np.float16)          # [DIN, SHARD]
        xtb = np.ascontiguousarray(
            xt.reshape(KI, 128, NCH, CHUNK)
            .transpose(2, 1, 0, 3).reshape(NCH * 128, KI * CHUNK))
        xtkb = np.ascontiguousarray(
            xt.reshape(KI, 128, NCH, CHUNK)
            .transpose(2, 0, 1, 3)[0:3].reshape(3 * KI * 128, CHUNK))
        in_maps.append(
            {
                "xt": xtb,
                "xtkb": xtkb,
                "w0": w0b,
                "w1": w1b,
                "w2": w2r,
                "b0": b0r,
                "b1": b1r,
                "b2": b2r,
            }
        )

    # The first execution of a freshly-compiled NEFF intermittently hits a
    # transient device error (NRT_EXEC_UNIT_UNRECOVERABLE); a retry succeeds.
    import time as _time

    last_err = None
    for _attempt in range(3):
        try:
            res = run_bass_kernel_spmd(nc, in_maps, core_ids=list(range(NCORES)))
            break
        except Exception as e:  # noqa: BLE001 - retry transient device faults
            last_err = e
            _time.sleep(3.0)
    else:
        raise last_err
    LAST_RESULTS = res

    out = np.concatenate([res.results[c]["out"][0] for c in range(NCORES)])
    alpha = np.concatenate([res.results[c]["alpha"][0] for c in range(NCORES)])
    return out[:, None].astype(np.float32), alpha[:, None].astype(np.float32)
